# revision 13
# baseline (speedup 1.0000x reference)
"""Trainium2 Bass kernel for nn_CrossmodalFusion (B=1024, R=36, D=1024).

Data-parallel over the batch dim across 8 NeuronCores. Each core processes
128 batches = 4608 region tokens. On-device activations are kept
feature-major (features on SBUF partitions, tokens on the free dim) so that
every matmul uses the (small, replicated) weights as the stationary lhsT
operand and no on-chip transposes of the big activations are needed. The
host passes each core's rgns slice pre-transposed to (D, tokens) and
transposes the result back; all FLOPs run on device.

Per-token attention scalars are produced as a (1, ntok) row via a
block-diagonal indicator matrix (affine_select) + ones-vector reduction on
the PE, and broadcast back to 128 partitions with a tiny DRAM bounce.
seg_rep ( = mask * q_b ) is folded into the last matmul by K-augmentation:
out += qw_local.T @ indicator, where qw = q @ f1_W is precomputed once.
"""
import os
import sys
import types
from contextlib import ExitStack

sys.path.insert(0, "/opt/trn_rl_repo")

import numpy as np

import concourse.bass as bass
import concourse.tile as tile
from concourse import bacc, mybir
from concourse.masks import make_identity

F32 = mybir.dt.float32
BF16 = mybir.dt.bfloat16
I32 = mybir.dt.int32

B, R, D = 1024, 36, 1024
H = D // 2
SEG_C = 133
NCORES = 8
BC = B // NCORES            # batches per core
TOK = BC * R                # tokens per core
NBT = 13                    # batches per token tile (13*36 = 468 <= 512)

KC = D // 128               # 8 feature chunks
KH = H // 128               # 4 hidden chunks

LAST_EXEC_NS = None


def _wire_ntff_hook():
    """bass_utils wants antenv.axon_hooks; the agent image lacks it. Provide
    an equivalent shim backed by the injected libaxon so trace=True works."""
    if "antenv.axon_hooks" in sys.modules:
        return
    try:
        import trn_agent_boot.trn_boot as tb
        hook = tb._ntff_profile_via_ctypes("/opt/axon/libaxon_pjrt.so")
    except Exception:
        hook = None
    mod = types.ModuleType("antenv.axon_hooks")
    _h = [hook]
    mod.set_axon_ntff_profile_hook = lambda h: _h.__setitem__(0, h)
    mod.get_axon_ntff_profile_hook = lambda: _h[0]
    sys.modules["antenv.axon_hooks"] = mod


def _tiles():
    """(b0, nb) batch-aligned token tiles covering BC batches."""
    out = []
    b0 = 0
    while b0 < BC:
        nb = min(NBT, BC - b0)
        out.append((b0, nb))
        b0 += nb
    return out


def _build(nc):
    ctx = ExitStack()
    with tile.TileContext(nc) as tc, ctx:
        _emit(ctx, tc)
    nc.compile()
    return nc


def _emit(ctx, tc):
    nc = tc.nc
    AF = mybir.ActivationFunctionType
    ALU = mybir.AluOpType

    # ---- DRAM I/O -------------------------------------------------------
    xT = nc.dram_tensor("xT", [D, TOK], F32, kind="ExternalInput").ap()
    unet = nc.dram_tensor("unet", [BC, SEG_C, 49], F32, kind="ExternalInput").ap()
    lens = nc.dram_tensor("lens", [1, BC], I32, kind="ExternalInput").ap()
    wi = {}
    for name, shape in [
        ("mi_W1", [D, H]), ("mi_b1", [1, H]), ("mi_W2", [H, D]), ("mi_b2", [1, D]),
        ("ms_W1", [D, H]), ("ms_b1", [1, H]), ("ms_W2", [H, D]), ("ms_b2", [1, D]),
        ("seg_W", [SEG_C, D]), ("seg_b", [1, D]), ("ln_g", [1, D]), ("ln_b", [1, D]),
        ("sc_W", [D, D]), ("sc_b", [1, D]), ("f1_W", [D, D]), ("f1_b", [1, D]),
    ]:
        wi[name] = nc.dram_tensor(name, shape, F32, kind="ExternalInput").ap()
    outT = nc.dram_tensor("outT", [D, TOK], F32, kind="ExternalOutput").ap()

    tls = _tiles()
    m_scr = nc.dram_tensor("m_scr", [1, TOK], BF16).ap()
    qw_scr = nc.dram_tensor("qw_scr", [BC, D], BF16).ap()
    w_scr = nc.dram_tensor("w_scr", [len(tls), 512], BF16).ap()

    # ---- persistent constants ------------------------------------------
    const = ctx.enter_context(tc.tile_pool(name="const", bufs=1))

    def load_w_bf(name, kchunks, m):
        ap_ = wi[name]
        t = const.tile([128, kchunks, m], BF16, tag=f"cw_{name}")
        nc.gpsimd.dma_start(t[:], ap_.rearrange("(kc p) m -> p kc m", p=128))
        return t

    W_mi1 = load_w_bf("mi_W1", KC, H)
    W_mi2 = load_w_bf("mi_W2", KH, D)
    W_sc = load_w_bf("sc_W", KC, D)
    W_f1 = load_w_bf("f1_W", KC, D)

    def load_col_f32(name, mchunks):
        # (1, mchunks*128) vector -> (128, mchunks) per-partition columns
        ap_ = wi[name]
        t = const.tile([128, mchunks], F32, tag=f"cc_{name}")
        src = bass.AP(tensor=ap_.tensor, offset=ap_.offset, ap=[[1, 128], [128, mchunks]])
        nc.gpsimd.dma_start(t[:], src)
        return t

    b_mi1c = load_col_f32("mi_b1", KH)
    b_scc = load_col_f32("sc_b", KC)
    b_f1c = load_col_f32("f1_b", KC)

    def load_row_bf(name, n):
        t = const.tile([1, n], BF16, tag=f"cr_{name}")
        nc.gpsimd.dma_start(t[:], wi[name])
        return t

    b_mi2r = load_row_bf("mi_b2", D)

    ones_row = const.tile([1, 512], BF16)
    nc.vector.memset(ones_row[:], 1.0)
    ones_col = const.tile([128, 1], BF16)
    nc.vector.memset(ones_col[:], 1.0)
    ident_bf = const.tile([128, 128], BF16)
    make_identity(nc, ident_bf)
    eps_t = const.tile([128, 1], F32)
    nc.vector.memset(eps_t[:], 1e-5)

    qT_bf = const.tile([128, KC, BC], BF16)    # feature-major q (lhsT for attn)
    qw_bf = const.tile([BC, D], BF16)          # token-major q @ f1_W

    psum = ctx.enter_context(tc.tile_pool(name="psum", bufs=1, space="PSUM"))
    psum_s = psum

    # ============================ q-stage ================================
    with tc.tile_pool(name="qpool", bufs=1) as qp:
        W_ms1 = qp.tile([128, KC, H], BF16)
        nc.gpsimd.dma_start(W_ms1[:], wi["ms_W1"].rearrange("(kc p) m -> p kc m", p=128))
        W_ms2 = qp.tile([128, KH, D], BF16)
        nc.gpsimd.dma_start(W_ms2[:], wi["ms_W2"].rearrange("(kc p) m -> p kc m", p=128))
        W_seg_a = qp.tile([128, D], BF16)
        nc.gpsimd.dma_start(W_seg_a[:], wi["seg_W"][0:128, :])
        W_seg_b = qp.tile([5, D], BF16)
        nc.gpsimd.dma_start(W_seg_b[:], wi["seg_W"][128:SEG_C, :])
        b_segr = qp.tile([1, D], BF16)
        nc.gpsimd.dma_start(b_segr[:], wi["seg_b"])
        b_ms1r = qp.tile([1, H], BF16)
        nc.gpsimd.dma_start(b_ms1r[:], wi["ms_b1"])
        b_ms2r = qp.tile([1, D], BF16)
        nc.gpsimd.dma_start(b_ms2r[:], wi["ms_b2"])
        g_bc = qp.tile([128, D], F32)
        nc.sync.dma_start(g_bc[:], bass.AP(tensor=wi["ln_g"].tensor, offset=wi["ln_g"].offset, ap=[[0, 128], [1, D]]))
        bb_bc = qp.tile([128, D], F32)
        nc.sync.dma_start(bb_bc[:], bass.AP(tensor=wi["ln_b"].tensor, offset=wi["ln_b"].offset, ap=[[0, 128], [1, D]]))

        # avgpool(7x7) -> pooledT (SEG_C, BC), computed channel-major directly
        unetT_a = qp.tile([128, BC, 49], F32)
        nc.sync.dma_start(unetT_a[:], unet.rearrange("b c j -> c b j")[0:128])
        unetT_b = qp.tile([5, BC, 49], F32)
        nc.sync.dma_start(unetT_b[:], unet.rearrange("b c j -> c b j")[128:SEG_C])
        pooledT_a = qp.tile([128, BC], F32)
        nc.vector.reduce_sum(pooledT_a[:], unetT_a[:], axis=mybir.AxisListType.X)
        pooledT_b = qp.tile([5, BC], F32)
        nc.vector.reduce_sum(pooledT_b[:], unetT_b[:], axis=mybir.AxisListType.X)
        pa_bf = qp.tile([128, BC], BF16)
        nc.scalar.mul(pa_bf[:], pooledT_a[:], 1.0 / 49.0)
        pb_bf = qp.tile([5, BC], BF16)
        nc.scalar.mul(pb_bf[:], pooledT_b[:], 1.0 / 49.0)

        # q1 = relu(pooled @ seg_W + seg_b)   (token-major: BC x D)
        q1 = qp.tile([BC, D], F32)
        for ng in range(2):
            sl = slice(ng * 512, (ng + 1) * 512)
            ps = psum.tile([BC, 512], F32, tag="mmps", bufs=4)
            nc.tensor.matmul(ps[:], pa_bf[:], W_seg_a[:, sl], start=True, stop=False)
            nc.tensor.matmul(ps[:], pb_bf[:], W_seg_b[:, sl], start=False, stop=False)
            nc.tensor.matmul(ps[:], ones_row[0:1, 0:BC], b_segr[0:1, sl], start=False, stop=True)
            nc.vector.tensor_scalar_max(q1[:, sl], ps[:], 0.0)

        # layernorm over D
        stats = qp.tile([BC, 2, 6], F32)
        for s in range(2):
            nc.vector.bn_stats(stats[:, s, :], q1[:, s * 512:(s + 1) * 512])
        mv = qp.tile([BC, 2], F32)
        nc.vector.bn_aggr(mv[:], stats[:])
        rstd = qp.tile([BC, 1], F32)
        nc.scalar.activation(rstd[:], mv[:, 1:2], AF.Sqrt, bias=eps_t[0:BC, :])
        nc.vector.reciprocal(rstd[:], rstd[:])
        qn = qp.tile([BC, D], F32)
        nc.vector.tensor_scalar(qn[:], q1[:], mv[:, 0:1], rstd[:],
                                op0=ALU.subtract, op1=ALU.mult)
        nc.vector.tensor_mul(qn[:], qn[:], g_bc[0:BC, :])
        qn_bf = qp.tile([BC, D], BF16)
        nc.vector.tensor_add(qn_bf[:], qn[:], bb_bc[0:BC, :])

        # qnT (feature-major) via PE transposes
        qnT_bf = qp.tile([128, KC, BC], BF16)
        for kc in range(KC):
            pt = psum_s.tile([128, BC], BF16, tag="tps", bufs=1)
            nc.tensor.transpose(pt[:], qn_bf[:, kc * 128:(kc + 1) * 128], ident_bf[0:BC, 0:BC])
            nc.scalar.copy(qnT_bf[:, kc, :], pt[:])

        # q MLP (feature-major): qm = relu(ms_W1.T @ qnT + b1)
        qmT_bf = qp.tile([128, KH, BC], BF16)
        for mc in range(KH):
            sl = slice(mc * 128, (mc + 1) * 128)
            ps = psum.tile([128, BC], F32, tag="mmps", bufs=4)
            for kc in range(KC):
                nc.tensor.matmul(ps[:], W_ms1[:, kc, sl], qnT_bf[:, kc, :],
                                 start=(kc == 0), stop=False)
            nc.tensor.matmul(ps[:], b_ms1r[0:1, sl], ones_row[0:1, 0:BC],
                             start=False, stop=True)
            nc.scalar.activation(qmT_bf[:, mc, :], ps[:], AF.Relu)
        # q2T = ms_W2.T @ qmT + b2 + qnT   -> qT_bf
        for mc in range(KC):
            sl = slice(mc * 128, (mc + 1) * 128)
            ps = psum.tile([128, BC], F32, tag="mmps", bufs=4)
            for kc in range(KH):
                nc.tensor.matmul(ps[:], W_ms2[:, kc, sl], qmT_bf[:, kc, :],
                                 start=(kc == 0), stop=False)
            nc.tensor.matmul(ps[:], b_ms2r[0:1, sl], ones_row[0:1, 0:BC],
                             start=False, stop=True)
            nc.vector.tensor_add(qT_bf[:, mc, :], ps[:], qnT_bf[:, mc, :])

        # qw = q2 @ f1_W (token-major, no f1_b)
        for ng in range(2):
            sl = slice(ng * 512, (ng + 1) * 512)
            ps = psum.tile([BC, 512], F32, tag="mmps", bufs=4)
            for kc in range(KC):
                nc.tensor.matmul(ps[:], qT_bf[:, kc, :], W_f1[:, kc, sl],
                                 start=(kc == 0), stop=(kc == KC - 1))
            nc.scalar.copy(qw_bf[:, sl], ps[:])
        nc.sync.dma_start(qw_scr[:, :], qw_bf[:])

        # token mask (BC, R): m[b, j] = j < lens[b]; store flat to m_scr
        j_i = qp.tile([BC, R], I32)
        nc.gpsimd.iota(j_i[:], pattern=[[1, R]], base=0, channel_multiplier=0)
        lens_col = qp.tile([BC, 1], I32)
        nc.gpsimd.dma_start(lens_col[:], bass.AP(tensor=lens.tensor, offset=lens.offset,
                                                 ap=[[1, BC], [1, 1]]))
        lens_f = qp.tile([BC, 1], F32)
        nc.vector.tensor_copy(lens_f[:], lens_col[:])
        j_f = qp.tile([BC, R], F32)
        nc.vector.tensor_copy(j_f[:], j_i[:])
        m_bf = qp.tile([BC, R], BF16)
        nc.vector.tensor_scalar(m_bf[:], j_f[:], lens_f[:], None, op0=ALU.is_lt)
        nc.sync.dma_start(m_scr[0:1, :].rearrange("o (b j) -> (o b) j", j=R), m_bf[:])

    # ============================ main loop ==============================
    xp = ctx.enter_context(tc.tile_pool(name="xp", bufs=2))
    hp = ctx.enter_context(tc.tile_pool(name="hp", bufs=2))
    rp = ctx.enter_context(tc.tile_pool(name="rp", bufs=2))
    wcp = ctx.enter_context(tc.tile_pool(name="wcp", bufs=2))
    scp = ctx.enter_context(tc.tile_pool(name="scp", bufs=2))
    zp = ctx.enter_context(tc.tile_pool(name="zp", bufs=2))
    op = ctx.enter_context(tc.tile_pool(name="op", bufs=2))
    sp = ctx.enter_context(tc.tile_pool(name="sp", bufs=2))

    xT_r = xT.rearrange("(kc p) t -> p kc t", p=128)
    outT_r = outT.rearrange("(kc p) t -> p kc t", p=128)

    for ti, (b0, nb) in enumerate(tls):
        nt = nb * R
        t0 = b0 * R

        x_bf = xp.tile([128, KC, nt], BF16, tag="x")
        nc.gpsimd.dma_start(x_bf[:], xT_r[:, :, t0:t0 + nt])

        m_bc = sp.tile([nb, nb, R], BF16, tag="mbc")
        nc.sync.dma_start(m_bc[:], bass.AP(tensor=m_scr.tensor, offset=m_scr.offset + t0,
                                           ap=[[0, nb], [1, nt]]))
        ind = sp.tile([nb, nb, R], BF16, tag="ind")
        nc.gpsimd.affine_select(out=ind[:], in_=m_bc[:], pattern=[[1, nb], [0, R]],
                                compare_op=ALU.is_equal, fill=0.0, base=0,
                                channel_multiplier=-1)
        ind_f = ind[:].rearrange("p s j -> p (s j)")

        # mm1: h1 = relu(mi_W1.T @ x + b1)
        h1_bf = hp.tile([128, KH, nt], BF16, tag="h1")
        for mc in range(KH):
            sl = slice(mc * 128, (mc + 1) * 128)
            ps = psum.tile([128, nt], F32, tag="mmps", bufs=4)
            for kc in range(KC):
                nc.tensor.matmul(ps[:], W_mi1[:, kc, sl], x_bf[:, kc, :],
                                 start=(kc == 0), stop=(kc == KC - 1))
            nc.scalar.activation(h1_bf[:, mc, :], ps[:], AF.Relu, bias=b_mi1c[:, mc:mc + 1])

        # mm2: r = mi_W2.T @ h1 + b2 + x
        r_bf = rp.tile([128, KC, nt], BF16, tag="r")
        for mc in range(KC):
            sl = slice(mc * 128, (mc + 1) * 128)
            ps = psum.tile([128, nt], F32, tag="mmps", bufs=4)
            for kc in range(KH):
                nc.tensor.matmul(ps[:], W_mi2[:, kc, sl], h1_bf[:, kc, :],
                                 start=(kc == 0), stop=False)
            nc.tensor.matmul(ps[:], b_mi2r[0:1, sl], ones_row[0:1, 0:nt],
                             start=False, stop=True)
            nc.vector.tensor_add(r_bf[:, mc, :], ps[:], x_bf[:, mc, :])

        # attention row: blockdiag(q_local @ r) * mask -> sigmoid -> w
        at = psum_s.tile([nb, nt], F32, tag="atps", bufs=2)
        for kc in range(KC):
            nc.tensor.matmul(at[:], qT_bf[:, kc, b0:b0 + nb], r_bf[:, kc, :],
                             start=(kc == 0), stop=(kc == KC - 1))
        masked = sp.tile([nb, nt], BF16, tag="msk")
        nc.vector.tensor_tensor(masked[:], at[:], ind_f, op=ALU.mult)
        ar = psum_s.tile([1, nt], F32, tag="arps", bufs=1)
        nc.tensor.matmul(ar[:], ones_col[0:nb, :], masked[:], start=True, stop=True)
        w_row = sp.tile([1, nt], F32, tag="wrow")
        nc.scalar.activation(w_row[:], ar[:], AF.Sigmoid, scale=float(1.0 / np.sqrt(D)))
        w_row_bf = sp.tile([1, nt], BF16, tag="wrowbf")
        m_row_f = m_bc[:].rearrange("p s j -> p (s j)")
        nc.vector.tensor_tensor(w_row_bf[:], w_row[:], m_row_f[0:1, :], op=ALU.mult)
        # broadcast w to 128 partitions via DRAM bounce
        nc.sync.dma_start(w_scr[ti:ti + 1, 0:nt], w_row_bf[:])
        w_bc = sp.tile([128, nt], BF16, tag="wbc")
        nc.sync.dma_start(w_bc[:], bass.AP(tensor=w_scr.tensor,
                                           offset=w_scr.offset + ti * 512,
                                           ap=[[0, 128], [1, nt]]))

        # wc = w * r ; mm3: scaling = tanh(sc_W.T @ wc + sc_b)
        wc_bf = wcp.tile([128, KC, nt], BF16, tag="wc")
        for kc in range(KC):
            nc.vector.tensor_mul(wc_bf[:, kc, :], r_bf[:, kc, :], w_bc[:])
        sc_bf = scp.tile([128, KC, nt], BF16, tag="sc")
        for mc in range(KC):
            sl = slice(mc * 128, (mc + 1) * 128)
            ps = psum.tile([128, nt], F32, tag="mmps", bufs=4)
            for kc in range(KC):
                nc.tensor.matmul(ps[:], W_sc[:, kc, sl], wc_bf[:, kc, :],
                                 start=(kc == 0), stop=(kc == KC - 1))
            nc.scalar.activation(sc_bf[:, mc, :], ps[:], AF.Tanh, bias=b_scc[:, mc:mc + 1])

        # z = wc * scaling ; mm4: out = relu(f1_W.T @ z + qw_local.T @ ind + f1_b)
        z_bf = zp.tile([128, KC, nt], BF16, tag="z")
        for kc in range(KC):
            nc.vector.tensor_mul(z_bf[:, kc, :], wc_bf[:, kc, :], sc_bf[:, kc, :])
        qw_loc = sp.tile([nb, D], BF16, tag="qwloc")
        nc.sync.dma_start(qw_loc[:], qw_scr[b0:b0 + nb, :])
        o_f = op.tile([128, KC, nt], F32, tag="o")
        for mc in range(KC):
            sl = slice(mc * 128, (mc + 1) * 128)
            ps = psum.tile([128, nt], F32, tag="mmps", bufs=4)
            for kc in range(KC):
                nc.tensor.matmul(ps[:], W_f1[:, kc, sl], z_bf[:, kc, :],
                                 start=(kc == 0), stop=False)
            nc.tensor.matmul(ps[:], qw_loc[:, sl], ind_f, start=False, stop=True)
            nc.scalar.activation(o_f[:, mc, :], ps[:], AF.Relu, bias=b_f1c[:, mc:mc + 1])
        nc.sync.dma_start(outT_r[:, :, t0:t0 + nt], o_f[:])


_NC_CACHE = {}


def _get_nc():
    if "nc" not in _NC_CACHE:
        nc = bacc.Bacc("TRN2", target_bir_lowering=False, debug=False)
        _NC_CACHE["nc"] = _build(nc)
    return _NC_CACHE["nc"]


def kernel(rgns, Unet_segs, region_lens, mi_W1, mi_b1, mi_W2, mi_b2,
           ms_W1, ms_b1, ms_W2, ms_b2, seg_W, seg_b, ln_g, ln_b,
           sc_W, sc_b, f1_W, f1_b):
    global LAST_EXEC_NS
    _wire_ntff_hook()
    from concourse.bass_utils import run_bass_kernel_spmd

    f = lambda a: np.ascontiguousarray(np.asarray(a, dtype=np.float32))
    rgns = f(rgns)
    unet = f(Unet_segs).reshape(B, SEG_C, 49)
    lens = np.ascontiguousarray(np.asarray(region_lens).astype(np.int32))

    weights = {
        "mi_W1": f(mi_W1), "mi_b1": f(mi_b1).reshape(1, H), "mi_W2": f(mi_W2),
        "mi_b2": f(mi_b2).reshape(1, D), "ms_W1": f(ms_W1), "ms_b1": f(ms_b1).reshape(1, H),
        "ms_W2": f(ms_W2), "ms_b2": f(ms_b2).reshape(1, D), "seg_W": f(seg_W),
        "seg_b": f(seg_b).reshape(1, D), "ln_g": f(ln_g).reshape(1, D),
        "ln_b": f(ln_b).reshape(1, D), "sc_W": f(sc_W), "sc_b": f(sc_b).reshape(1, D),
        "f1_W": f(f1_W), "f1_b": f(f1_b).reshape(1, D),
    }

    rflat = rgns.reshape(B * R, D)
    in_maps = []
    for c in range(NCORES):
        sl = slice(c * TOK, (c + 1) * TOK)
        in_maps.append(dict(
            xT=np.ascontiguousarray(rflat[sl].T),
            unet=np.ascontiguousarray(unet[c * BC:(c + 1) * BC]),
            lens=np.ascontiguousarray(lens[c * BC:(c + 1) * BC].reshape(1, BC)),
            **weights,
        ))

    nc = _get_nc()
    trace = bool(int(os.environ.get("BASSK_TRACE", "0")))
    res = run_bass_kernel_spmd(nc, in_maps, list(range(NCORES)), trace=trace)
    LAST_EXEC_NS = res.exec_time_ns

    out = np.empty((B * R, D), np.float32)
    for c in range(NCORES):
        out[c * TOK:(c + 1) * TOK] = res.results[c]["outT"].T
    return out.reshape(B, R, D)
